# revision 42
# baseline (speedup 1.0000x reference)
"""Trainium2 Bass kernel for nn_BehlerG2 (Behler-style angular symmetry functions).

Final design (~51.7us on TRN2, vs 106us baseline):
- 8 cores; core c handles batch b = c // 2, atom half h = c % 2 (128 atoms/core,
  one atom per SBUF partition, Tp = 290 compacted triple slots).
- Host compacts each atom's triple list by mask, pads with dummy entry 256
  (coords 1e4, beyond cutoff; z_ijk = 0 kills it exactly), sorts canonical
  order by j, and pre-subtracts p_i into a per-partition fp16 coordinate
  table, so scattered values are D = p_neighbor - p_i directly.
- Gather: three GPSIMD local_scatters moving fp16 coords (1 int16 per field,
  half the index count of f32):
    scat_k: k-table -> KS (k-sorted)   [first; ~3.5us ucode + ~4us store-drain]
    scat_j: j-table -> FJ (canonical)
    perm:   KS -> KF (canonical)       [targets the first-scattered side so
                                        the j fill overlaps the perm ucode]
  The ucode overlay is preloaded; a tiny warmup scatter absorbs first-use cost.
- Run expansion: one tensor_tensor_scan per coord plane
  (state = gapmask*state + val). Gap masks are HOST-BUILT (exact run
  structure): value-based detection would misfire on D == 0 (j == i),
  which are large contributors.
- Geometry in fp16 intermediates (2x DVE): r2 planes, S3, and the
  dot-product cross term CD = Dj.Dk (r_jk^2 = r_ij^2 + r_ik^2 - 2*CD,
  NUM = 2*CD free, TB = RR - CD). r_jk^2 clamped >= 0 (fp16-rounded squares
  can push exactly-coincident j==k slightly negative -> sqrt NaN otherwise).
- Window: r = sqrt(r2 + 1e-12) on ACT; fc product via sin(pi/12*min(r,6)+pi/2)
  with one squaring; cutoff handled by sin(pi) ~ 0 and clamping base1 to
  [-2, 2] (valid triples lie in [0, 2] by Cauchy-Schwarz, which bounds
  dead-triple cos garbage through base1^8). 1/(rij*rik) via DVE reciprocal
  (ACT ln/exp thrashes activation tables; Pool tensor_scalar is ~12x slow).
- Pair stage: E[e,z,t] = U_e[t]*H_z[t] with U_e = exp(-eta_e*S3) (8 ACT exps,
  pinned after Sin by a zero bias derived from CS so the exp-table load is
  not hoisted), H = W*BQ as one 2x fp16 TT (powers of base1 by repeated
  squaring into one BQ tile; fp16-overflow scales folded into output weights).
  Reductions: 20 scalar_tensor_tensor+accum_out on DVE and 12 ACT
  Copy-with-accum over 2x fp16 product planes (both lanes co-terminate).
  Heavy GPSIMD work is kept off this phase (V ops crawl next to Pool ucode).
"""

import sys

if "/opt/trn_rl_repo" not in sys.path:
    sys.path.insert(0, "/opt/trn_rl_repo")

import numpy as np

import concourse.bacc as bacc
import concourse.mybir as mybir
import concourse.tile as tile
from concourse import library_config
from concourse.alu_op_type import AluOpType as alu
from concourse.bass_utils import run_bass_kernel_spmd

f32 = mybir.dt.float32
f16 = mybir.dt.float16
i16 = mybir.dt.int16

B, A, T = 4, 256, 512
NCORES = 8
P = 128          # atoms per core == partitions
NE = 258         # table entries: 256 atoms + dummy(256) + spare(257)
ZETAS = [1.0, 2.0, 4.0, 8.0]
ZSC = [1.0, 1.0, 4.0, 64.0]   # fp16-overflow scale folded out of H4/H8
CUTOFF = 6.0
PI = float(np.pi)
FAR = 1.0e4      # dummy-entry coordinate (beyond cutoff)

AF = mybir.ActivationFunctionType


def _build_program(Tp: int, etas: np.ndarray):
    assert Tp % 2 == 0 and 6 * Tp * 32 < 2**16, f"Tp={Tp}"

    nc = bacc.Bacc("TRN2", target_bir_lowering=False, debug=False, num_devices=NCORES)

    tabb_d = nc.dram_tensor("tabb", [P, 3 * NE], i16, kind="ExternalInput")
    kixb_d = nc.dram_tensor("kixb", [P, 3 * NE], i16, kind="ExternalInput")
    bj_d = nc.dram_tensor("bj", [P, 3 * NE + 5 * Tp], i16, kind="ExternalInput")
    bw_d = nc.dram_tensor("bw", [P, Tp + 64], f32, kind="ExternalInput")
    out_d = nc.dram_tensor("out", [P, 64], f32, kind="ExternalOutput")

    with tile.TileContext(nc) as tc:
        with tc.tile_pool(name="main", bufs=1) as pool:
            # preload the local_scatter ucode overlay while inputs stream in
            nc.gpsimd.load_library(library_config.local_scatter)

            # tiny warmup scatter: absorbs first-use cost on the Pool queue
            WSRC = pool.tile([P, 2], i16)
            WDST = pool.tile([P, 2], i16)
            nc.vector.memset(WSRC, 0)
            WIX = pool.tile([P, 2], i16)
            nc.vector.memset(WIX, -1)
            nc.gpsimd.local_scatter(WDST, WSRC, WIX, channels=P,
                                    num_elems=2, num_idxs=2)

            # parallel DMA queues: scat_k's deps split across sync + scalar
            TABi = pool.tile([P, 3 * NE], i16)
            nc.sync.dma_start(TABi, tabb_d.ap())
            KIX = pool.tile([P, 3 * NE], i16)
            nc.scalar.dma_start(KIX, kixb_d.ap())
            BJ = pool.tile([P, 3 * NE + 5 * Tp], i16)
            nc.scalar.dma_start(BJ, bj_d.ap())
            BW = pool.tile([P, Tp + 64], f32)
            nc.sync.dma_start(BW, bw_d.ap())

            JIX = BJ[:, : 3 * NE]
            PIX = BJ[:, 3 * NE : 3 * NE + 3 * Tp]
            MJM = BJ[:, 3 * NE + 3 * Tp : 3 * NE + 4 * Tp].bitcast(f16)
            MKM = BJ[:, 3 * NE + 4 * Tp :].bitcast(f16)
            ZIJK = BW[:, 0:Tp]
            CLO = BW[:, Tp : Tp + 32]
            CHI = BW[:, Tp + 32 :]

            EPS = pool.tile([P, 1], f32)
            nc.vector.memset(EPS, 1e-12)
            HPI = pool.tile([P, 1], f32)
            nc.vector.memset(HPI, PI / 2.0)
            TINY = pool.tile([P, 2], f32)
            # preload the sqrt activation table while scatters run
            nc.scalar.activation(TINY, EPS.broadcast_to([P, 2]), AF.Sqrt)

            # separate side tiles (a shared tile would create false deps
            # between the perm write and j-side reads)
            FJ = pool.tile([P, 3, Tp], f16)   # j side, canonical (fp16 coords)
            KF = pool.tile([P, 3, Tp], f16)   # k side, canonical (perm target)
            KS = pool.tile([P, 3, Tp], f16)   # k side, k-sorted order

            def scat(out3, data, idx, nidx):
                nc.gpsimd.local_scatter(
                    out3.bitcast(i16).rearrange("p a b -> p (a b)"),
                    data, idx, channels=P, num_elems=3 * Tp, num_idxs=nidx,
                )

            def fill(SIDE, MASK):
                # The host pre-subtracts p_i into the per-partition fp16
                # table, so scattered values are D = p_neighbor - p_i.  The
                # gap mask (1 = copy previous slot) comes from the host, which
                # knows the exact run structure -- value-based gap detection
                # would misfire on D == 0 (j == i), which are big contributors.
                for q in range(3):
                    nc.vector.tensor_tensor_scan(
                        out=SIDE[:, q], data0=MASK, data1=SIDE[:, q],
                        initial=0.0, op0=alu.mult, op1=alu.add,
                    )

            # ---- front-end ----
            scat(KS, TABi, KIX, 3 * NE)      # first: k side (will be permuted)
            fill(KS, MKM)
            scat(FJ, TABi, JIX, 3 * NE)      # j side lands directly canonical
            scat(KF, KS.bitcast(i16).rearrange("p a b -> p (a b)"), PIX, 3 * Tp)
            fill(FJ, MJM)

            # j-side geometry while the perm runs (FJ/KF already hold Dj/Dk)
            DSQ = pool.tile([P, 3, Tp], f16)
            R2 = pool.tile([P, 3, Tp], f16)   # [rij2, rik2, rjk2]
            TMP = pool.tile([P, Tp], f16)
            nc.vector.tensor_tensor(out=DSQ, in0=FJ, in1=FJ, op=alu.mult)
            nc.vector.tensor_tensor(out=TMP, in0=DSQ[:, 0], in1=DSQ[:, 1], op=alu.add)
            nc.vector.tensor_tensor(out=R2[:, 0], in0=TMP, in1=DSQ[:, 2], op=alu.add)

            # k side + cross terms after the perm, all on V (a V/Pool split
            # here loses: the first consumer of the perm-written tile pays the
            # ucode store-drain on either engine, and concurrent wide V+Pool
            # TTs slow each other ~1.6x).  Cross term via the dot product:
            # r_jk^2 = r_ij^2 + r_ik^2 - 2*Dj.Dk, and NUM = 2*CD for free.
            U3 = pool.tile([P, 3, Tp], f16)
            PT = pool.tile([P, Tp], f16)
            CD = pool.tile([P, Tp], f16)
            nc.vector.tensor_tensor(out=U3, in0=FJ, in1=KF, op=alu.mult)
            nc.vector.tensor_tensor(out=PT, in0=U3[:, 0], in1=U3[:, 1], op=alu.add)
            nc.vector.tensor_tensor(out=CD, in0=PT, in1=U3[:, 2], op=alu.add)
            nc.vector.tensor_tensor(out=DSQ, in0=KF, in1=KF, op=alu.mult)
            nc.vector.tensor_tensor(out=TMP, in0=DSQ[:, 0], in1=DSQ[:, 1], op=alu.add)
            nc.vector.tensor_tensor(out=R2[:, 1], in0=TMP, in1=DSQ[:, 2], op=alu.add)

            # r_jk^2 from the dot product (completes the sqrt input)
            SP0 = pool.tile([P, Tp], f16)
            nc.vector.tensor_tensor(out=SP0, in0=R2[:, 0], in1=R2[:, 1], op=alu.add)
            nc.vector.scalar_tensor_tensor(out=R2[:, 2], in0=CD, scalar=-2.0,
                                           in1=SP0, op0=alu.mult, op1=alu.add)
            nc.vector.tensor_scalar(out=R2[:, 2], in0=R2[:, 2], scalar1=0.0,
                                    scalar2=None, op0=alu.max)

            # ---- scalar geometry ----
            R = pool.tile([P, 3, Tp], f32)
            Rf = R.rearrange("p a s -> p (a s)")
            nc.scalar.activation(Rf, R2.rearrange("p a s -> p (a s)"), AF.Sqrt, bias=EPS)

            RR = pool.tile([P, Tp], f32)
            nc.vector.tensor_tensor(out=RR, in0=R[:, 0], in1=R[:, 1], op=alu.mult)
            nc.vector.tensor_scalar(out=Rf, in0=Rf, scalar1=CUTOFF, scalar2=None, op0=alu.min)
            CS = pool.tile([P, 3, Tp], f32)
            nc.scalar.activation(CS.rearrange("p a s -> p (a s)"), Rf,
                                 AF.Sin, scale=PI / 12.0, bias=HPI)

            # zero bias derived from CS: pins every Exp after Sin on ACT.
            # Early in V's order so the exps launch as soon as Sin lands.
            SINB = pool.tile([P, 1], f32)
            nc.vector.tensor_scalar(out=SINB, in0=CS[:, 0, 0:1], scalar1=0.0,
                                    scalar2=None, op0=alu.mult)

            S3 = pool.tile([P, Tp], f16)
            nc.vector.tensor_tensor(out=S3, in0=SP0, in1=R2[:, 2], op=alu.add)
            # TB = RR - 0.5*NUM = RR - CD
            TB = pool.tile([P, Tp], f32)
            nc.vector.tensor_tensor(out=TB, in0=RR, in1=CD, op=alu.subtract)
            RCP = pool.tile([P, Tp], f32)
            nc.vector.reciprocal(RCP, RR)

            # W = (cs_ij*cs_ik*cs_jk)^2 * z_ijk on Pool (overlaps V's B chain)
            CP = pool.tile([P, Tp], f32)
            W = pool.tile([P, Tp], f16)
            nc.gpsimd.tensor_tensor(out=CP, in0=CS[:, 0], in1=CS[:, 1], op=alu.mult)
            nc.gpsimd.tensor_tensor(out=CP, in0=CP, in1=CS[:, 2], op=alu.mult)
            nc.gpsimd.tensor_tensor(out=CP, in0=CP, in1=CP, op=alu.mult)
            nc.gpsimd.tensor_tensor(out=W, in0=CP, in1=ZIJK, op=alu.mult)

            # base1 = 1 - cos(theta) = (RR - 0.5*NUM)/RR, clamped to [-2, 2];
            # all base powers land in one BQ tile so H is a single wide TT
            BQ = pool.tile([P, 4, Tp], f16)
            nc.vector.tensor_tensor(out=BQ[:, 0], in0=TB, in1=RCP, op=alu.mult)
            nc.vector.tensor_scalar(out=BQ[:, 0], in0=BQ[:, 0], scalar1=2.0,
                                    scalar2=-2.0, op0=alu.min, op1=alu.max)
            nc.vector.tensor_tensor(out=BQ[:, 1], in0=BQ[:, 0], in1=BQ[:, 0], op=alu.mult)
            nc.vector.scalar_tensor_tensor(out=BQ[:, 2], in0=BQ[:, 1], scalar=0.25,
                                           in1=BQ[:, 1], op0=alu.mult, op1=alu.mult)
            nc.vector.scalar_tensor_tensor(out=BQ[:, 3], in0=BQ[:, 2], scalar=0.25,
                                           in1=BQ[:, 2], op0=alu.mult, op1=alu.mult)

            UE = pool.tile([P, 8, Tp], f16)
            for e in range(8):
                nc.scalar.activation(UE[:, e], S3, AF.Exp, scale=float(-etas[e]),
                                     bias=SINB)

            H = pool.tile([P, 4, Tp], f16)
            WB = W.rearrange("p (a t) -> p a t", a=1).broadcast_to([P, 4, Tp])
            nc.vector.tensor_tensor(out=H, in0=WB, in1=BQ, op=alu.mult)

            # ---- pair stage: PART[e,z] = sum_t U_e * H_z, all on DVE ----
            PART = pool.tile([P, 32], f32)
            SCR0 = pool.tile([P, Tp], f16)
            SCR1 = pool.tile([P, Tp], f16)
            SCR = [SCR0, SCR1]
            # first NACT etas: V forms one bunched product plane per eta
            # (cheaper than 4 STTs), idle ACT reduces via in-place Copy-accum
            NACT = 3
            PRE = pool.tile([P, NACT, 4, Tp], f16)
            for e in range(NACT):
                UB = UE[:, e].rearrange("p (a t) -> p a t", a=1).broadcast_to([P, 4, Tp])
                nc.vector.tensor_tensor(out=PRE[:, e], in0=UB, in1=H, op=alu.mult)
                for z in range(4):
                    idx = e * 4 + z
                    nc.scalar.activation(PRE[:, e, z], PRE[:, e, z], AF.Copy,
                                         accum_out=PART[:, idx : idx + 1])
            # V lane: bunched 2x fp16 product + fp16-out 2x tensor_reduce
            # (zijk carries 2^-8 so the fp16 sums cannot overflow)
            PART16 = pool.tile([P, 32], f16)
            PV = pool.tile([P, 4, Tp], f16)
            with nc.allow_low_precision("fp16 pair sums; zijk pre-scaled 2^-8"):
                for e in range(NACT, 8):
                    UB2 = UE[:, e].rearrange("p (a t) -> p a t", a=1).broadcast_to([P, 4, Tp])
                    nc.vector.tensor_tensor(out=PV, in0=UB2, in1=H, op=alu.mult)
                    nc.vector.tensor_reduce(
                        out=PART16[:, e * 4 : (e + 1) * 4], in_=PV,
                        axis=mybir.AxisListType.X, op=alu.add,
                    )

            # ---- final scaling into [128, 64] ----
            OUT = pool.tile([P, 64], f32)
            Ov = OUT.rearrange("p (e g z) -> p e g z", e=8, g=2, z=4)
            Pv = PART.rearrange("p (e z) -> p e z", e=8, z=4)
            Qv = PART16.rearrange("p (e z) -> p e z", e=8, z=4)
            Lv = CLO.rearrange("p (e z) -> p e z", e=8, z=4)
            Hv = CHI.rearrange("p (e z) -> p e z", e=8, z=4)
            na = NACT
            nc.vector.tensor_tensor(out=Ov[:, :na, 0], in0=Pv[:, :na], in1=Lv[:, :na], op=alu.mult)
            nc.vector.tensor_tensor(out=Ov[:, :na, 1], in0=Pv[:, :na], in1=Hv[:, :na], op=alu.mult)
            nc.vector.tensor_tensor(out=Ov[:, na:, 0], in0=Qv[:, na:], in1=Lv[:, na:], op=alu.mult)
            nc.vector.tensor_tensor(out=Ov[:, na:, 1], in0=Qv[:, na:], in1=Hv[:, na:], op=alu.mult)
            nc.sync.dma_start(out_d.ap(), OUT)

    nc.compile()
    return nc


def _first_occurrence_slots(sorted_vals: np.ndarray) -> np.ndarray:
    """sorted_vals [P, Tp] ascending. Returns [P, NE] int64: first slot of
    each entry value, -1 if absent."""
    Pn, Tpn = sorted_vals.shape
    fm = np.ones((Pn, Tpn), dtype=bool)
    fm[:, 1:] = sorted_vals[:, 1:] != sorted_vals[:, :-1]
    idx = np.full((Pn, NE), -1, np.int64)
    pp, ss = np.nonzero(fm)
    idx[pp, sorted_vals[pp, ss]] = ss
    return idx


def _table_idx(idx_slots: np.ndarray, Tp: int) -> np.ndarray:
    """idx_slots [P, NE] (slot or -1) -> int16 [P, 3*NE]: for fp16 data
    element i = f*NE + e, destination = f*Tp + slot[e] (or -1)."""
    s = idx_slots[:, None, :]                            # [P,1,NE]
    f = np.arange(3)[None, :, None]
    arr = np.where(s >= 0, f * Tp + s, -1)
    return arr.reshape(P, 3 * NE).astype(np.int16)


def _prepare_host(inputs):
    positions = np.asarray(inputs["positions"], dtype=np.float32)
    nj = np.asarray(inputs["neighbors_j"]).astype(np.int64)
    nk = np.asarray(inputs["neighbors_k"]).astype(np.int64)
    mask = np.asarray(inputs["mask_triples"]) != 0
    atomic = np.asarray(inputs["atomic_numbers"]).astype(np.float32)
    etas = np.asarray(inputs["etas"], dtype=np.float32)

    assert not np.any(positions == 0.0), "scan gap-mask relies on nonzero coords"

    counts = mask.sum(axis=2)
    Tp = int(counts.max())
    Tp = max(16, Tp + (Tp & 1))   # round up to even only
    assert 6 * Tp * 32 < 2**16, f"Tp={Tp} too large for merged local_scatter"

    order = np.argsort(~mask, axis=2, kind="stable")
    valid = np.take_along_axis(mask, order, 2)[:, :, :Tp]
    jpad = np.where(valid, np.take_along_axis(nj, order, 2)[:, :, :Tp], 256)
    kpad = np.where(valid, np.take_along_axis(nk, order, 2)[:, :, :Tp], 256)

    clo_row = np.array([(2.0 ** (1.0 - zv)) * sc * 256.0 for _ in range(8)
                        for zv, sc in zip(ZETAS, ZSC)], dtype=np.float32)
    chi_row = np.array([(2.0 ** (1.0 + zv)) * sc * 256.0 for _ in range(8)
                        for zv, sc in zip(ZETAS, ZSC)], dtype=np.float32)

    in_maps = []
    for c in range(NCORES):
        b, h = divmod(c, 2)
        asl = slice(h * P, (h + 1) * P)
        jp = jpad[b, asl]    # [P, Tp]
        kp = kpad[b, asl]

        # canonical order: j-sorted.  k side built k-sorted then permuted.
        jorder = np.argsort(jp, axis=1, kind="stable")
        jcan = np.take_along_axis(jp, jorder, 1)
        kcan = np.take_along_axis(kp, jorder, 1)
        korder = np.argsort(kcan, axis=1, kind="stable")
        ksorted = np.take_along_axis(kcan, korder, 1)

        jix = _table_idx(_first_occurrence_slots(jcan), Tp)
        kix = _table_idx(_first_occurrence_slots(ksorted), Tp)

        # perm: fp16 data element i = f*Tp + t -> dst f*Tp + korder[t]
        s = korder[:, None, :]
        f = np.arange(3)[None, :, None]
        pix = (f * Tp + s).reshape(P, 3 * Tp).astype(np.int16)

        # per-partition fp16 coordinate table, pre-subtracted: entry e of
        # partition p holds p_e - p_atom(p)
        fars = np.full(NE - 256, FAR, np.float32)
        tab3 = np.empty((P, 3, NE), np.float32)
        for q in range(3):
            base = np.concatenate([positions[b, :, q], fars]).astype(np.float32)
            tab3[:, q, :] = base[None, :] - positions[b, asl, q][:, None]
        tab_i16 = np.ascontiguousarray(
            tab3.astype(np.float16).reshape(P, 3 * NE)).view(np.int16)

        # z_ijk per canonical slot (dummy entries -> 0)
        z258 = np.concatenate([atomic[b], np.zeros(NE - 256, np.float32)])
        # 2^-8 keeps fp16 PART sums below 65504 (compensated in clo/chi)
        zijk = ((z258[jcan] * z258[kcan]) * (1.0 / 256.0)).astype(np.float32)

        def gapmask(sorted_ids):
            gm = np.zeros(sorted_ids.shape, np.float16)
            gm[:, 1:] = (sorted_ids[:, 1:] == sorted_ids[:, :-1]).astype(np.float16)
            return np.ascontiguousarray(gm).view(np.int16)

        bj = np.concatenate([jix, pix, gapmask(jcan), gapmask(ksorted)], axis=1)
        bw = np.concatenate([
            zijk,
            np.broadcast_to(clo_row, (P, 32)),
            np.broadcast_to(chi_row, (P, 32)),
        ], axis=1).astype(np.float32)
        in_maps.append({"tabb": tab_i16, "kixb": kix, "bj": bj, "bw": bw})

    return Tp, etas, in_maps


def kernel(**inputs) -> np.ndarray:
    Tp, etas, in_maps = _prepare_host(inputs)
    nc = _build_program(Tp, etas)
    res = run_bass_kernel_spmd(nc, in_maps, core_ids=list(range(NCORES)))
    out = np.zeros((B, A, 64), np.float32)
    for c in range(NCORES):
        b, h = divmod(c, 2)
        out[b, h * P : (h + 1) * P] = res.results[c]["out"]
    return out


# revision 43
# speedup vs baseline: 1.0676x; 1.0676x over previous
"""Trainium2 Bass kernel for nn_BehlerG2 (Behler-style angular symmetry functions).

Final design (~51.7us on TRN2, vs 106us baseline):
- 8 cores; core c handles batch b = c // 2, atom half h = c % 2 (128 atoms/core,
  one atom per SBUF partition, Tp = 290 compacted triple slots).
- Host compacts each atom's triple list by mask, pads with dummy entry 256
  (coords 1e4, beyond cutoff; z_ijk = 0 kills it exactly), sorts canonical
  order by j, and pre-subtracts p_i into a per-partition fp16 coordinate
  table, so scattered values are D = p_neighbor - p_i directly.
- Gather: three GPSIMD local_scatters moving fp16 coords (1 int16 per field,
  half the index count of f32):
    scat_k: k-table -> KS (k-sorted)   [first; ~3.5us ucode + ~4us store-drain]
    scat_j: j-table -> FJ (canonical)
    perm:   KS -> KF (canonical)       [targets the first-scattered side so
                                        the j fill overlaps the perm ucode]
  The ucode overlay is preloaded; a tiny warmup scatter absorbs first-use cost.
- Run expansion: one tensor_tensor_scan per coord plane
  (state = gapmask*state + val). Gap masks are HOST-BUILT (exact run
  structure): value-based detection would misfire on D == 0 (j == i),
  which are large contributors.
- Geometry in fp16 intermediates (2x DVE): r2 planes, S3, and the
  dot-product cross term CD = Dj.Dk (r_jk^2 = r_ij^2 + r_ik^2 - 2*CD,
  NUM = 2*CD free, TB = RR - CD). r_jk^2 clamped >= 0 (fp16-rounded squares
  can push exactly-coincident j==k slightly negative -> sqrt NaN otherwise).
- Window: r = sqrt(r2 + 1e-12) on ACT; fc product via sin(pi/12*min(r,6)+pi/2)
  with one squaring; cutoff handled by sin(pi) ~ 0 and clamping base1 to
  [-2, 2] (valid triples lie in [0, 2] by Cauchy-Schwarz, which bounds
  dead-triple cos garbage through base1^8). 1/(rij*rik) via DVE reciprocal
  (ACT ln/exp thrashes activation tables; Pool tensor_scalar is ~12x slow).
- Pair stage: E[e,z,t] = U_e[t]*H_z[t] with U_e = exp(-eta_e*S3) (8 ACT exps,
  pinned after Sin by a zero bias derived from CS so the exp-table load is
  not hoisted), H = W*BQ as one 2x fp16 TT (powers of base1 by repeated
  squaring into one BQ tile; fp16-overflow scales folded into output weights).
  Reductions: 20 scalar_tensor_tensor+accum_out on DVE and 12 ACT
  Copy-with-accum over 2x fp16 product planes (both lanes co-terminate).
  Heavy GPSIMD work is kept off this phase (V ops crawl next to Pool ucode).
"""

import sys

if "/opt/trn_rl_repo" not in sys.path:
    sys.path.insert(0, "/opt/trn_rl_repo")

import numpy as np

import concourse.bacc as bacc
import concourse.mybir as mybir
import concourse.tile as tile
from concourse import library_config
from concourse.alu_op_type import AluOpType as alu
from concourse.bass_utils import run_bass_kernel_spmd

f32 = mybir.dt.float32
f16 = mybir.dt.float16
i16 = mybir.dt.int16

B, A, T = 4, 256, 512
NCORES = 8
P = 128          # atoms per core == partitions
NE = 258         # table entries: 256 atoms + dummy(256) + spare(257)
ZETAS = [1.0, 2.0, 4.0, 8.0]
ZSC = [1.0, 1.0, 4.0, 64.0]   # fp16-overflow scale folded out of H4/H8
CUTOFF = 6.0
PI = float(np.pi)
FAR = 1.0e4      # dummy-entry coordinate (beyond cutoff)

AF = mybir.ActivationFunctionType


def _build_program(Tp: int, etas: np.ndarray):
    assert Tp % 2 == 0 and 6 * Tp * 32 < 2**16, f"Tp={Tp}"

    nc = bacc.Bacc("TRN2", target_bir_lowering=False, debug=False, num_devices=NCORES)

    tabb_d = nc.dram_tensor("tabb", [P, 3 * NE], i16, kind="ExternalInput")
    kixb_d = nc.dram_tensor("kixb", [P, 3 * NE], i16, kind="ExternalInput")
    bj_d = nc.dram_tensor("bj", [P, 3 * NE + 5 * Tp], i16, kind="ExternalInput")
    bw_d = nc.dram_tensor("bw", [P, Tp + 64], f32, kind="ExternalInput")
    out_d = nc.dram_tensor("out", [P, 64], f32, kind="ExternalOutput")

    with tile.TileContext(nc) as tc:
        with tc.tile_pool(name="main", bufs=1) as pool:
            # preload the local_scatter ucode overlay while inputs stream in
            nc.gpsimd.load_library(library_config.local_scatter)

            # tiny warmup scatter: absorbs first-use cost on the Pool queue
            WSRC = pool.tile([P, 2], i16)
            WDST = pool.tile([P, 2], i16)
            nc.vector.memset(WSRC, 0)
            WIX = pool.tile([P, 2], i16)
            nc.vector.memset(WIX, -1)
            nc.gpsimd.local_scatter(WDST, WSRC, WIX, channels=P,
                                    num_elems=2, num_idxs=2)

            # parallel DMA queues: scat_k's deps split across sync + scalar
            TABi = pool.tile([P, 3 * NE], i16)
            nc.sync.dma_start(TABi, tabb_d.ap())
            KIX = pool.tile([P, 3 * NE], i16)
            nc.scalar.dma_start(KIX, kixb_d.ap())
            BJ = pool.tile([P, 3 * NE + 5 * Tp], i16)
            nc.scalar.dma_start(BJ, bj_d.ap())
            BW = pool.tile([P, Tp + 64], f32)
            nc.sync.dma_start(BW, bw_d.ap())

            JIX = BJ[:, : 3 * NE]
            PIX = BJ[:, 3 * NE : 3 * NE + 3 * Tp]
            MJM = BJ[:, 3 * NE + 3 * Tp : 3 * NE + 4 * Tp].bitcast(f16)
            MKM = BJ[:, 3 * NE + 4 * Tp :].bitcast(f16)
            ZIJK = BW[:, 0:Tp]
            CLO = BW[:, Tp : Tp + 32]
            CHI = BW[:, Tp + 32 :]

            EPS = pool.tile([P, 1], f32)
            nc.vector.memset(EPS, 1e-12)
            HPI = pool.tile([P, 1], f32)
            nc.vector.memset(HPI, PI / 2.0)
            TINY = pool.tile([P, 2], f32)
            # preload the sqrt activation table while scatters run
            nc.scalar.activation(TINY, EPS.broadcast_to([P, 2]), AF.Sqrt)

            # separate side tiles (a shared tile would create false deps
            # between the perm write and j-side reads)
            FJ = pool.tile([P, 3, Tp], f16)   # j side, canonical (fp16 coords)
            KF = pool.tile([P, 3, Tp], f16)   # k side, canonical (perm target)
            KS = pool.tile([P, 3, Tp], f16)   # k side, k-sorted order

            def scat(out3, data, idx, nidx):
                nc.gpsimd.local_scatter(
                    out3.bitcast(i16).rearrange("p a b -> p (a b)"),
                    data, idx, channels=P, num_elems=3 * Tp, num_idxs=nidx,
                )

            def fill(SIDE, MASK):
                # The host pre-subtracts p_i into the per-partition fp16
                # table, so scattered values are D = p_neighbor - p_i.  The
                # gap mask (1 = copy previous slot) comes from the host, which
                # knows the exact run structure -- value-based gap detection
                # would misfire on D == 0 (j == i), which are big contributors.
                for q in range(3):
                    nc.vector.tensor_tensor_scan(
                        out=SIDE[:, q], data0=MASK, data1=SIDE[:, q],
                        initial=0.0, op0=alu.mult, op1=alu.add,
                    )

            # ---- front-end ----
            scat(KS, TABi, KIX, 3 * NE)      # first: k side (will be permuted)
            fill(KS, MKM)
            scat(FJ, TABi, JIX, 3 * NE)      # j side lands directly canonical
            scat(KF, KS.bitcast(i16).rearrange("p a b -> p (a b)"), PIX, 3 * Tp)
            fill(FJ, MJM)

            # j-side geometry while the perm runs (FJ/KF already hold Dj/Dk)
            DSQ = pool.tile([P, 3, Tp], f16)
            R2 = pool.tile([P, 3, Tp], f16)   # [rij2, rik2, rjk2]
            TMP = pool.tile([P, Tp], f16)
            nc.vector.tensor_tensor(out=DSQ, in0=FJ, in1=FJ, op=alu.mult)
            nc.vector.tensor_tensor(out=TMP, in0=DSQ[:, 0], in1=DSQ[:, 1], op=alu.add)
            nc.vector.tensor_tensor(out=R2[:, 0], in0=TMP, in1=DSQ[:, 2], op=alu.add)

            # k side + cross terms after the perm, all on V (a V/Pool split
            # here loses: the first consumer of the perm-written tile pays the
            # ucode store-drain on either engine, and concurrent wide V+Pool
            # TTs slow each other ~1.6x).  Cross term via the dot product:
            # r_jk^2 = r_ij^2 + r_ik^2 - 2*Dj.Dk, and NUM = 2*CD for free.
            U3 = pool.tile([P, 3, Tp], f16)
            PT = pool.tile([P, Tp], f16)
            CD = pool.tile([P, Tp], f16)
            nc.vector.tensor_tensor(out=U3, in0=FJ, in1=KF, op=alu.mult)
            nc.vector.tensor_tensor(out=PT, in0=U3[:, 0], in1=U3[:, 1], op=alu.add)
            nc.vector.tensor_tensor(out=CD, in0=PT, in1=U3[:, 2], op=alu.add)
            nc.vector.tensor_tensor(out=DSQ, in0=KF, in1=KF, op=alu.mult)
            nc.vector.tensor_tensor(out=TMP, in0=DSQ[:, 0], in1=DSQ[:, 1], op=alu.add)
            nc.vector.tensor_tensor(out=R2[:, 1], in0=TMP, in1=DSQ[:, 2], op=alu.add)

            # r_jk^2 from the dot product (completes the sqrt input)
            SP0 = pool.tile([P, Tp], f16)
            nc.vector.tensor_tensor(out=SP0, in0=R2[:, 0], in1=R2[:, 1], op=alu.add)
            nc.vector.scalar_tensor_tensor(out=R2[:, 2], in0=CD, scalar=-2.0,
                                           in1=SP0, op0=alu.mult, op1=alu.add)
            nc.vector.tensor_scalar(out=R2[:, 2], in0=R2[:, 2], scalar1=0.0,
                                    scalar2=None, op0=alu.max)

            # ---- scalar geometry ----
            R = pool.tile([P, 3, Tp], f32)
            Rf = R.rearrange("p a s -> p (a s)")
            nc.scalar.activation(Rf, R2.rearrange("p a s -> p (a s)"), AF.Sqrt, bias=EPS)

            RR = pool.tile([P, Tp], f32)
            nc.vector.tensor_tensor(out=RR, in0=R[:, 0], in1=R[:, 1], op=alu.mult)
            nc.vector.tensor_scalar(out=Rf, in0=Rf, scalar1=CUTOFF, scalar2=None, op0=alu.min)
            CS = pool.tile([P, 3, Tp], f32)
            nc.scalar.activation(CS.rearrange("p a s -> p (a s)"), Rf,
                                 AF.Sin, scale=PI / 12.0, bias=HPI)

            # zero bias derived from CS: pins every Exp after Sin on ACT.
            # Early in V's order so the exps launch as soon as Sin lands.
            SINB = pool.tile([P, 1], f32)
            nc.vector.tensor_scalar(out=SINB, in0=CS[:, 0, 0:1], scalar1=0.0,
                                    scalar2=None, op0=alu.mult)

            S3 = pool.tile([P, Tp], f16)
            nc.vector.tensor_tensor(out=S3, in0=SP0, in1=R2[:, 2], op=alu.add)
            # TB = RR - 0.5*NUM = RR - CD
            TB = pool.tile([P, Tp], f32)
            nc.vector.tensor_tensor(out=TB, in0=RR, in1=CD, op=alu.subtract)
            RCP = pool.tile([P, Tp], f32)
            nc.vector.reciprocal(RCP, RR)

            # W = (cs_ij*cs_ik*cs_jk)^2 * z_ijk on Pool (overlaps V's B chain)
            CP = pool.tile([P, Tp], f32)
            W = pool.tile([P, Tp], f16)
            nc.gpsimd.tensor_tensor(out=CP, in0=CS[:, 0], in1=CS[:, 1], op=alu.mult)
            nc.gpsimd.tensor_tensor(out=CP, in0=CP, in1=CS[:, 2], op=alu.mult)
            nc.gpsimd.tensor_tensor(out=CP, in0=CP, in1=CP, op=alu.mult)
            nc.gpsimd.tensor_tensor(out=W, in0=CP, in1=ZIJK, op=alu.mult)

            # base1 = 1 - cos(theta) = (RR - 0.5*NUM)/RR, clamped to [-2, 2];
            # all base powers land in one BQ tile so H is a single wide TT
            BQ = pool.tile([P, 4, Tp], f16)
            nc.vector.tensor_tensor(out=BQ[:, 0], in0=TB, in1=RCP, op=alu.mult)
            nc.vector.tensor_scalar(out=BQ[:, 0], in0=BQ[:, 0], scalar1=2.0,
                                    scalar2=-2.0, op0=alu.min, op1=alu.max)
            nc.vector.tensor_tensor(out=BQ[:, 1], in0=BQ[:, 0], in1=BQ[:, 0], op=alu.mult)
            nc.vector.scalar_tensor_tensor(out=BQ[:, 2], in0=BQ[:, 1], scalar=0.25,
                                           in1=BQ[:, 1], op0=alu.mult, op1=alu.mult)
            nc.vector.scalar_tensor_tensor(out=BQ[:, 3], in0=BQ[:, 2], scalar=0.25,
                                           in1=BQ[:, 2], op0=alu.mult, op1=alu.mult)

            UE = pool.tile([P, 8, Tp], f16)
            for e in range(8):
                nc.scalar.activation(UE[:, e], S3, AF.Exp, scale=float(-etas[e]),
                                     bias=SINB)

            H = pool.tile([P, 4, Tp], f16)
            WB = W.rearrange("p (a t) -> p a t", a=1).broadcast_to([P, 4, Tp])
            nc.vector.tensor_tensor(out=H, in0=WB, in1=BQ, op=alu.mult)

            # ---- pair stage: PART[e,z] = sum_t U_e * H_z, all on DVE ----
            PART = pool.tile([P, 32], f32)
            SCR0 = pool.tile([P, Tp], f16)
            SCR1 = pool.tile([P, Tp], f16)
            SCR = [SCR0, SCR1]
            # first NACT etas: V forms one bunched product plane per eta
            # (cheaper than 4 STTs), idle ACT reduces via in-place Copy-accum
            NACT = 3
            PRE = pool.tile([P, NACT, 4, Tp], f16)
            for e in range(NACT):
                UB = UE[:, e].rearrange("p (a t) -> p a t", a=1).broadcast_to([P, 4, Tp])
                nc.vector.tensor_tensor(out=PRE[:, e], in0=UB, in1=H, op=alu.mult)
                for z in range(4):
                    idx = e * 4 + z
                    nc.scalar.activation(PRE[:, e, z], PRE[:, e, z], AF.Copy,
                                         accum_out=PART[:, idx : idx + 1])
            for e in range(NACT, 8):
                for z in range(4):
                    idx = e * 4 + z
                    nc.vector.scalar_tensor_tensor(
                        out=SCR[idx % 2], in0=UE[:, e], scalar=1.0, in1=H[:, z],
                        op0=alu.mult, op1=alu.mult,
                        accum_out=PART[:, idx : idx + 1],
                    )

            # ---- final scaling into [128, 64] ----
            OUT = pool.tile([P, 64], f32)
            Ov = OUT.rearrange("p (e g z) -> p e g z", e=8, g=2, z=4)
            Pv = PART.rearrange("p (e z) -> p e z", e=8, z=4)
            Lv = CLO.rearrange("p (e z) -> p e z", e=8, z=4)
            Hv = CHI.rearrange("p (e z) -> p e z", e=8, z=4)
            nc.vector.tensor_tensor(out=Ov[:, :, 0], in0=Pv, in1=Lv, op=alu.mult)
            nc.vector.tensor_tensor(out=Ov[:, :, 1], in0=Pv, in1=Hv, op=alu.mult)
            nc.sync.dma_start(out_d.ap(), OUT)

    nc.compile()
    return nc


def _first_occurrence_slots(sorted_vals: np.ndarray) -> np.ndarray:
    """sorted_vals [P, Tp] ascending. Returns [P, NE] int64: first slot of
    each entry value, -1 if absent."""
    Pn, Tpn = sorted_vals.shape
    fm = np.ones((Pn, Tpn), dtype=bool)
    fm[:, 1:] = sorted_vals[:, 1:] != sorted_vals[:, :-1]
    idx = np.full((Pn, NE), -1, np.int64)
    pp, ss = np.nonzero(fm)
    idx[pp, sorted_vals[pp, ss]] = ss
    return idx


def _table_idx(idx_slots: np.ndarray, Tp: int) -> np.ndarray:
    """idx_slots [P, NE] (slot or -1) -> int16 [P, 3*NE]: for fp16 data
    element i = f*NE + e, destination = f*Tp + slot[e] (or -1)."""
    s = idx_slots[:, None, :]                            # [P,1,NE]
    f = np.arange(3)[None, :, None]
    arr = np.where(s >= 0, f * Tp + s, -1)
    return arr.reshape(P, 3 * NE).astype(np.int16)


def _prepare_host(inputs):
    positions = np.asarray(inputs["positions"], dtype=np.float32)
    nj = np.asarray(inputs["neighbors_j"]).astype(np.int64)
    nk = np.asarray(inputs["neighbors_k"]).astype(np.int64)
    mask = np.asarray(inputs["mask_triples"]) != 0
    atomic = np.asarray(inputs["atomic_numbers"]).astype(np.float32)
    etas = np.asarray(inputs["etas"], dtype=np.float32)

    assert not np.any(positions == 0.0), "scan gap-mask relies on nonzero coords"

    counts = mask.sum(axis=2)
    Tp = int(counts.max())
    Tp = max(16, Tp + (Tp & 1))   # round up to even only
    assert 6 * Tp * 32 < 2**16, f"Tp={Tp} too large for merged local_scatter"

    order = np.argsort(~mask, axis=2, kind="stable")
    valid = np.take_along_axis(mask, order, 2)[:, :, :Tp]
    jpad = np.where(valid, np.take_along_axis(nj, order, 2)[:, :, :Tp], 256)
    kpad = np.where(valid, np.take_along_axis(nk, order, 2)[:, :, :Tp], 256)

    clo_row = np.array([(2.0 ** (1.0 - zv)) * sc for _ in range(8)
                        for zv, sc in zip(ZETAS, ZSC)], dtype=np.float32)
    chi_row = np.array([(2.0 ** (1.0 + zv)) * sc for _ in range(8)
                        for zv, sc in zip(ZETAS, ZSC)], dtype=np.float32)

    in_maps = []
    for c in range(NCORES):
        b, h = divmod(c, 2)
        asl = slice(h * P, (h + 1) * P)
        jp = jpad[b, asl]    # [P, Tp]
        kp = kpad[b, asl]

        # canonical order: j-sorted.  k side built k-sorted then permuted.
        jorder = np.argsort(jp, axis=1, kind="stable")
        jcan = np.take_along_axis(jp, jorder, 1)
        kcan = np.take_along_axis(kp, jorder, 1)
        korder = np.argsort(kcan, axis=1, kind="stable")
        ksorted = np.take_along_axis(kcan, korder, 1)

        jix = _table_idx(_first_occurrence_slots(jcan), Tp)
        kix = _table_idx(_first_occurrence_slots(ksorted), Tp)

        # perm: fp16 data element i = f*Tp + t -> dst f*Tp + korder[t]
        s = korder[:, None, :]
        f = np.arange(3)[None, :, None]
        pix = (f * Tp + s).reshape(P, 3 * Tp).astype(np.int16)

        # per-partition fp16 coordinate table, pre-subtracted: entry e of
        # partition p holds p_e - p_atom(p)
        fars = np.full(NE - 256, FAR, np.float32)
        tab3 = np.empty((P, 3, NE), np.float32)
        for q in range(3):
            base = np.concatenate([positions[b, :, q], fars]).astype(np.float32)
            tab3[:, q, :] = base[None, :] - positions[b, asl, q][:, None]
        tab_i16 = np.ascontiguousarray(
            tab3.astype(np.float16).reshape(P, 3 * NE)).view(np.int16)

        # z_ijk per canonical slot (dummy entries -> 0)
        z258 = np.concatenate([atomic[b], np.zeros(NE - 256, np.float32)])
        zijk = (z258[jcan] * z258[kcan]).astype(np.float32)

        def gapmask(sorted_ids):
            gm = np.zeros(sorted_ids.shape, np.float16)
            gm[:, 1:] = (sorted_ids[:, 1:] == sorted_ids[:, :-1]).astype(np.float16)
            return np.ascontiguousarray(gm).view(np.int16)

        bj = np.concatenate([jix, pix, gapmask(jcan), gapmask(ksorted)], axis=1)
        bw = np.concatenate([
            zijk,
            np.broadcast_to(clo_row, (P, 32)),
            np.broadcast_to(chi_row, (P, 32)),
        ], axis=1).astype(np.float32)
        in_maps.append({"tabb": tab_i16, "kixb": kix, "bj": bj, "bw": bw})

    return Tp, etas, in_maps


def kernel(**inputs) -> np.ndarray:
    Tp, etas, in_maps = _prepare_host(inputs)
    nc = _build_program(Tp, etas)
    res = run_bass_kernel_spmd(nc, in_maps, core_ids=list(range(NCORES)))
    out = np.zeros((B, A, 64), np.float32)
    for c in range(NCORES):
        b, h = divmod(c, 2)
        out[b, h * P : (h + 1) * P] = res.results[c]["out"]
    return out
